# revision 41
# baseline (speedup 1.0000x reference)
"""TRN2 Bass kernel for nn_MultiBlockStructuredScoreNet (neuron-major + fp8).

Measured 66.9-68.0us HW exec (baseline 78.3us), rel err 1.236e-2 (< 2e-2).

Computes s(z) = -grad_z U(z) for
  U(z) = sum_k MLP_k(z_k) + sum_r z_8^T W_r z_{8-r}
z: (8192, 9*256) f32, data-parallel over 8 NeuronCores (1024 samples each).

v2 design: every gradient matmul is emitted with the OUTPUT in neuron-major
orientation ([neuron partitions, sample free-dim]) so the W matrices are the
PE-stationary operand (loaded once, streamed against 512-sample tiles):
 - d_lag (8 blocks): fp8e4 DoubleRow matmuls -- W_r stationary with K=256
   (both neuron halves of z_future per pass), z_future pairs streamed.
   2x fewer PE cycles than fp16.  Measured end-to-end rel err 1.24e-2
   (budget 2e-2); the fp8 path is exact-deterministic vs the CPU model.
 - d_fut (block 8): fp16 (fp8 here would cost 3.1e-2 rel err), as one
   K=2048 GEMM over the 16 lag z-chunks, W^T tiles stationary.
 - MLP forward/backward: same h-major scheme as v1 (col-tiled u1, block-diag
   u2 / dh1 with -gW3 folded); d_z contributions are K=32 row-positioned
   matmuls accumulating into the same neuron-major PSUM banks.
All cross/dz weights carry a x64 scale (keeps fp8 W normals); the kernel
stores 64*score in fp16 and the host divides by 64 after the gather.
z ships fp16 for lag blocks (8192 cols/tile) + fp8 pairs for z_future;
output ships fp16.  ~11.7MB HBM traffic/core vs 17.7MB in v1.
"""

import numpy as np
import ml_dtypes

import concourse.bass as bass
import concourse.tile as tile
from concourse import bacc, mybir
from concourse.bass_utils import run_bass_kernel_spmd

AF = mybir.ActivationFunctionType
F32 = mybir.dt.float32
F16 = mybir.dt.float16
F8 = mybir.dt.float8e4
DR = mybir.MatmulPerfMode.DoubleRow

N_CORES = 8
BATCH = 8192
B_CORE = BATCH // N_CORES     # 1024
BT = 512                      # batch tile (PSUM free-dim max for f32)
NBT = B_CORE // BT            # 2 batch tiles per core
NB = 9                        # blocks
P_MAX = 8
NN = 256                      # neurons per block
D = NB * NN                   # 2304
H = 32
NZC = 2 * P_MAX               # fp16 zt chunks per batch tile (lag blocks)
ZW = NZC * BT                 # 8192

# pa (fp16 params) column layout
OFF_U1 = 0                    # 16 chunks x 32 cols (u1 lhsT, blocks 0-7)
OFF_W2 = 16 * 32              # 512: 3 x 128 block-diag u2 weights
OFF_W2T = OFF_W2 + 3 * 128    # 896: dh1 weights, -8*gW3 folded
PW = OFF_W2T + 3 * 128        # 1280

WSCALE = 64.0                 # fp8/cross weight scale, undone on host
DSCALE = 8.0                  # du1 and W1T each carry x8 (8*8 = 64)
MM_MODE = "fp16"              # unused; kept for test.py compat
WARMUP_MMS = 16


def _body(tc, out, zt, z8p, wd, wf, pa, pw1, biases, ctx):
    nc = tc.nc

    const = ctx.enter_context(tc.tile_pool(name="const", bufs=1))
    ztp = ctx.enter_context(tc.tile_pool(name="ztp", bufs=2))
    mlpp = ctx.enter_context(tc.tile_pool(name="mlpp", bufs=3, space="PSUM"))
    actp = ctx.enter_context(tc.tile_pool(name="actp", bufs=8))
    du1p = ctx.enter_context(tc.tile_pool(name="du1p", bufs=6))
    outp = ctx.enter_context(tc.tile_pool(name="outp", bufs=5, space="PSUM"))
    outsp = ctx.enter_context(tc.tile_pool(name="outs", bufs=2))

    # ---- inputs -> SBUF, ordered by first use
    pa_sb = const.tile([128, PW], F16, name="pa")
    nc.sync.dma_start(pa_sb[:], pa[:])
    bias_sb = const.tile([128, 6], F32, name="biassb")
    nc.sync.dma_start(bias_sb[:], biases[:])
    zt_sb = [ztp.tile([128, ZW], F16, tag="zt", name="ztsb") for _ in range(NBT)]
    nc.sync.dma_start(zt_sb[0][:, 0:8 * BT], zt[0, :, 0:8 * BT])
    wd_sb = const.tile([128, 2, 17, 128], F8, name="wd")
    nc.sync.dma_start(wd_sb[:], wd[:])
    z8_sb = [const.tile([128, 2, BT], F8, name=f"z8sb{i}") for i in range(NBT)]
    nc.sync.dma_start(z8_sb[0][:], z8p[0])
    nc.sync.dma_start(z8_sb[1][:], z8p[1])
    wf_sb = const.tile([128, 16, 2, 128], F16, name="wf")
    nc.sync.dma_start(wf_sb[:], wf[:])
    nc.sync.dma_start(zt_sb[0][:, 8 * BT:ZW], zt[0, :, 8 * BT:ZW])
    pw1_sb = const.tile([128, 18 * 128], F8, name="pw1")
    nc.sync.dma_start(pw1_sb[:], pw1[:])
    for a, b in [(0, 8 * BT), (8 * BT, ZW)]:
        nc.sync.dma_start(zt_sb[1][:, a:b], zt[1, :, a:b])

    outs = [outsp.tile([128, 18 * BT], F16, tag="outs", name="outst")
            for _ in range(NBT)]

    # ---- HAM warm-up on a memset tile: PE busy before any DMA lands
    scr2 = const.tile([128, 8], F16, name="scr2")
    wusrc = const.tile([128, BT], F16, name="wusrc")
    nc.gpsimd.memset(wusrc[:], 0.0)
    wu = outp.tile([128, BT], F32, tag="outp", name="wut")
    for _ in range(WARMUP_MMS):
        nc.tensor.matmul(wu[:], wusrc[:, 0:128], wusrc[:], start=True, stop=True)

    def bridge(n):
        # throwaway matmuls emitted into known DMA-wait holes so the PE
        # clock gate (HAM) stays open; they depend only on wusrc
        bw = outp.tile([128, BT], F32, tag="outp", name="bwt")
        for _ in range(n):
            nc.tensor.matmul(bw[:], wusrc[:, 0:128], wusrc[:],
                             start=True, stop=True)

    u1_state = {}
    act_state = {}
    du1_tiles = {}
    bwd_state = {}
    _ce = [0]

    def copy_eng(dst, src):
        # rotate PSUM->SBUF copies between DVE and ACT (2 DVE : 1 ACT --
        # ACT also carries the 9 activations per tile)
        _ce[0] += 1
        if _ce[0] % 3 == 0:
            nc.scalar.activation(dst, src, AF.Copy)
        else:
            nc.vector.tensor_copy(dst, src)

    st = {}

    def mlp_u1(t, g):
        u1 = mlpp.tile([128, BT], F32, tag="mlpp", name="u1t")
        if g < 2:
            P = 128
            for j in range(4):
                k = 4 * g + j
                for hf in range(2):
                    c = 2 * k + hf
                    nc.tensor.matmul(
                        u1[32 * j:32 * j + 32, :],
                        pa_sb[:, OFF_U1 + 32 * c:OFF_U1 + 32 * c + 32],
                        zt_sb[t][:, BT * c:BT * (c + 1)],
                        start=(hf == 0), stop=(hf == 1),
                        tile_position=(0, 32 * j))
        else:
            P = 32
            for hf in range(2):
                nc.tensor.matmul(
                    u1[0:32, :], wd_sb[:, hf:hf + 1, 16:17, 0:32],
                    z8_sb[t][:, hf:hf + 1, :],
                    start=(hf == 0), stop=(hf == 1), tile_position=(0, 0))
        st[(t, g)] = {"u1": u1, "P": P, "sc": 1.0 / WSCALE if g == 2 else 1.0}

    def act_h1(t, g):
        s = st[(t, g)]
        P = s["P"]
        h1 = actp.tile([128, BT], F16, tag="act", name="h1t")
        nc.scalar.activation(h1[:P], s["u1"][:P], AF.Silu,
                             bias=bias_sb[:P, g:g + 1], scale=s["sc"])
        s["h1"] = h1

    def act_sp1(t, g):
        s = st[(t, g)]
        P = s["P"]
        sp1 = actp.tile([128, BT], F16, tag="act", name="sp1t")
        nc.scalar.activation(sp1[:P], s["u1"][:P], AF.Derivative_silu,
                             bias=bias_sb[:P, g:g + 1], scale=s["sc"])
        s["sp1"] = sp1

    def mlp_u2(t, g):
        s = st[(t, g)]
        P = s["P"]
        u2 = mlpp.tile([128, BT], F32, tag="mlpp", name="u2t")
        nc.tensor.matmul(u2[:P], pa_sb[:P, OFF_W2 + 128 * g:OFF_W2 + 128 * g + P],
                         s["h1"][:P], start=True, stop=True)
        s["u2"] = u2

    def act_sp2(t, g):
        s = st[(t, g)]
        P = s["P"]
        sp2 = actp.tile([128, BT], F16, tag="act", name="sp2t")
        nc.scalar.activation(sp2[:P], s["u2"][:P], AF.Derivative_silu,
                             bias=bias_sb[:P, 3 + g:4 + g])
        s["sp2"] = sp2

    def mlp_dh1(t, g):
        s = st[(t, g)]
        P = s["P"]
        dh1 = mlpp.tile([128, BT], F32, tag="mlpp", name="dh1t")
        nc.tensor.matmul(dh1[:P],
                         pa_sb[:P, OFF_W2T + 128 * g:OFF_W2T + 128 * g + P],
                         sp2 := s["sp2"][:P], start=True, stop=True)
        du1 = du1p.tile([128, BT], F8, tag="du1", name="du1t")
        nc.vector.tensor_mul(du1[:P], dh1[:P], s["sp1"][:P])
        du1_tiles.setdefault(t, [None] * 3)[g] = du1

    def mlp_rest(t, g):
        # g1 runs standalone (its zt chunks arrive late): 2 table loads
        act_h1(t, g)
        act_sp1(t, g)
        mlp_u2(t, g)
        act_sp2(t, g)
        mlp_dh1(t, g)

    def dz_mm(t, op, k, nh):
        # MLP d_z for (block k, neuron half nh), accumulated last into op
        strip = 32 * (k % 4)
        cc = 128 * (2 * k + nh)
        nc.tensor.matmul(op[:], pw1_sb[strip:strip + 32, cc:cc + 128],
                         du1_tiles[t][k // 4][strip:strip + 32, :],
                         start=False, stop=True, tile_position=(strip, 0))

    def dr_mm(t, op, r, ih):
        ri = 2 * (r - 1) + ih
        nc.tensor.matmul(op[:], wd_sb[:, :, ri:ri + 1, :], z8_sb[t][:],
                         start=True, stop=False, perf_mode=DR)

    def fut_mm(t, o8, c, jh):
        nc.tensor.matmul(o8[:], wf_sb[:, c:c + 1, jh:jh + 1, :],
                         zt_sb[t][:, BT * c:BT * (c + 1)],
                         start=(c == 0), stop=False)

    def copy_out(t, op, c):
        copy_eng(outs[t][:, BT * c:BT * (c + 1)], op[:])

    def store(t, c0, c1):
        nc.gpsimd.dma_start(out[t, :, BT * c0:BT * c1],
                            outs[t][:, BT * c0:BT * c1])

    fut_bank = {}

    def cross_sb(t, sb, mid=None):
        # superblock: 2 lags (4 DR matmuls) + 8 interleaved d_fut chain
        # matmuls + 4 dz closers (adjacent lags -> alternating row strips
        # run pairwise-concurrent) + 4 copies.  The fut chain keeps the PE
        # streaming while DVE/ACT drain the lag banks.
        r0, r1 = 8 - 2 * sb, 7 - 2 * sb
        jh, ch0 = sb // 2, 8 * (sb % 2)
        if sb % 2 == 0:
            fut_bank[(t, jh)] = outp.tile([128, BT], F32, tag="outp",
                                          name="futpt")
        o8 = fut_bank[(t, jh)]
        ops = {}
        for r in (r0, r1):
            for ih in range(2):
                ops[(r, ih)] = outp.tile([128, BT], F32, tag="outp",
                                         name="outpt")
                dr_mm(t, ops[(r, ih)], r, ih)
        for c in range(ch0, ch0 + 8):
            fut_mm(t, o8, c, jh)
        if mid is not None:
            mid()
        for ih in range(2):
            for r in (r0, r1):
                dz_mm(t, ops[(r, ih)], 8 - r, ih)
        for r in (r0, r1):
            for ih in range(2):
                copy_out(t, ops[(r, ih)], 2 * (8 - r) + ih)
        if sb % 2 == 1:
            dz_mm(t, o8, 8, jh)
            copy_out(t, o8, 16 + jh)

    # ---- schedule: acts batched by function (Silu<->Dsilu switch costs a
    # 1.54us ACT table load); groups 0+2 pipeline together, g1 rides the
    # superblock mids when its zt chunks land.
    nc.scalar.activation(scr2[:, 0:8], wusrc[:, 0:8], AF.Silu)
    mlp_u1(0, 0)
    mlp_u1(0, 2)
    bridge(5)
    act_h1(0, 0)
    act_h1(0, 2)
    act_sp1(0, 0)
    act_sp1(0, 2)
    mlp_u2(0, 0)
    mlp_u2(0, 2)
    act_sp2(0, 0)
    act_sp2(0, 2)
    mlp_dh1(0, 0)
    mlp_dh1(0, 2)
    cross_sb(0, 0)
    cross_sb(0, 1, mid=lambda: mlp_u1(0, 1))
    store(0, 0, 8)
    store(0, 16, 17)
    cross_sb(0, 2, mid=lambda: mlp_rest(0, 1))
    cross_sb(0, 3)
    store(0, 8, 16)
    mlp_u1(1, 0)
    mlp_u1(1, 2)
    bridge(6)
    act_h1(1, 0)
    act_h1(1, 2)
    act_sp1(1, 0)
    act_sp1(1, 2)
    mlp_u2(1, 0)
    mlp_u2(1, 2)
    act_sp2(1, 0)
    act_sp2(1, 2)
    mlp_dh1(1, 0)
    mlp_dh1(1, 2)
    store(0, 17, 18)
    cross_sb(1, 0, mid=lambda: mlp_u1(1, 1))
    cross_sb(1, 1, mid=lambda: mlp_rest(1, 1))
    store(1, 0, 8)
    store(1, 16, 17)
    cross_sb(1, 2)
    store(1, 8, 12)
    cross_sb(1, 3)
    store(1, 12, 16)
    store(1, 17, 18)


# ------------------------------------------------------------- build + launch

_CACHED = {}


def _build():
    if "v2" in _CACHED:
        return _CACHED["v2"]
    nc = bacc.Bacc("TRN2", target_bir_lowering=False, debug=False,
                   num_devices=N_CORES)
    zt = nc.dram_tensor("zt", [NBT, 128, ZW], F16, kind="ExternalInput").ap()
    z8p = nc.dram_tensor("z8p", [NBT, 128, 2, BT], F8, kind="ExternalInput").ap()
    wd = nc.dram_tensor("wd", [128, 2, 17, 128], F8, kind="ExternalInput").ap()
    wf = nc.dram_tensor("wf", [128, 16, 2, 128], F16, kind="ExternalInput").ap()
    pa = nc.dram_tensor("pa", [128, PW], F16, kind="ExternalInput").ap()
    pw1 = nc.dram_tensor("pw1", [128, 18 * 128], F8, kind="ExternalInput").ap()
    biases = nc.dram_tensor("biases", [128, 6], F32, kind="ExternalInput").ap()
    out = nc.dram_tensor("out", [NBT, 128, 18 * BT], F16,
                         kind="ExternalOutput").ap()

    from contextlib import ExitStack
    with tile.TileContext(nc) as tc:
        with ExitStack() as ctx:
            _body(tc, out, zt, z8p, wd, wf, pa, pw1, biases, ctx)
    nc.compile()
    _CACHED["v2"] = nc
    return nc


def _prep_params(gW1, gb1, gW2, gb2, gW3, gb3, W):
    S = WSCALE
    pa = np.zeros((128, PW), np.float32)
    pw1 = np.zeros((128, 18 * 128), np.float32)
    biases = np.zeros((128, 6), np.float32)
    for k in range(8):
        for hf in range(2):
            c = 2 * k + hf
            pa[:, OFF_U1 + 32 * c:OFF_U1 + 32 * c + 32] = \
                gW1[k, 128 * hf:128 * (hf + 1), :]
    for k in range(NB):
        g, j = k // 4, k % 4
        rs = slice(32 * j, 32 * j + 32)
        pa[rs, OFF_W2 + 128 * g + 32 * j:OFF_W2 + 128 * g + 32 * j + 32] = gW2[k]
        # dh1 lhsT[g', h] = -gW3[g'] * gW2[h, g']
        pa[rs, OFF_W2T + 128 * g + 32 * j:OFF_W2T + 128 * g + 32 * j + 32] = \
            -DSCALE * gW3[k][:, None] * gW2[k].T
        biases[rs, g] = gb1[k]
        biases[rs, 3 + g] = gb2[k]
        strip = 32 * (k % 4)
        for nh in range(2):
            cc = 128 * (2 * k + nh)
            pw1[strip:strip + 32, cc:cc + 128] = \
                DSCALE * gW1[k, 128 * nh:128 * (nh + 1), :].T

    wd = np.zeros((128, 2, 17, 128), np.float32)
    for r in range(1, P_MAX + 1):
        for kt in range(2):
            for ih in range(2):
                wd[:, kt, 2 * (r - 1) + ih, :] = \
                    -S * W[r - 1][128 * kt:128 * (kt + 1), 128 * ih:128 * (ih + 1)]
    for kt in range(2):
        wd[:, kt, 16, 0:32] = S * gW1[8, 128 * kt:128 * (kt + 1), :]

    wf = np.zeros((128, 16, 2, 128), np.float32)
    for c in range(16):
        bc, hc = c // 2, c % 2
        for jh in range(2):
            wf[:, c, jh, :] = \
                -S * W[7 - bc][128 * jh:128 * (jh + 1),
                               128 * hc:128 * (hc + 1)].T
    return {"pa": pa.astype(np.float16),
            "pw1": pw1.astype(ml_dtypes.float8_e4m3),
            "wd": wd.astype(ml_dtypes.float8_e4m3),
            "wf": wf.astype(np.float16),
            "biases": biases}


def run(inputs, trace=False):
    nc = _build()
    params = _prep_params(
        np.asarray(inputs["gW1"]), np.asarray(inputs["gb1"]),
        np.asarray(inputs["gW2"]), np.asarray(inputs["gb2"]),
        np.asarray(inputs["gW3"]), np.asarray(inputs["gb3"]),
        np.asarray(inputs["W"]))
    z = np.asarray(inputs["z"])
    in_maps = []
    for ci in range(N_CORES):
        zc = z[ci * B_CORE:(ci + 1) * B_CORE]
        # zt[t, p, BT*c + s] = zc[BT*t + s, 128c + p] for lag blocks
        ztc = np.ascontiguousarray(
            zc[:, :NZC * 128].reshape(NBT, BT, NZC, 128).transpose(0, 3, 2, 1)
        ).reshape(NBT, 128, ZW).astype(np.float16)
        # z8p[t, p, kt, s] = zc[BT*t + s, 2048 + 128*kt + p]
        z8c = np.ascontiguousarray(
            zc[:, NZC * 128:].reshape(NBT, BT, 2, 128).transpose(0, 3, 2, 1)
        ).astype(ml_dtypes.float8_e4m3)
        in_maps.append({"zt": ztc, "z8p": z8c, **params})
    res = run_bass_kernel_spmd(nc, in_maps, core_ids=list(range(N_CORES)),
                               trace=trace)
    outs = []
    for r in res.results:
        o = r["out"].astype(np.float32) / WSCALE
        outs.append(o.reshape(NBT, 128, 18, BT).transpose(0, 3, 2, 1)
                    .reshape(B_CORE, D))
    return np.concatenate(outs, axis=0), res


def kernel(**inputs) -> np.ndarray:
    out, _ = run(inputs, trace=False)
    return out


# revision 42
# speedup vs baseline: 1.0392x; 1.0392x over previous
"""TRN2 Bass kernel for nn_MultiBlockStructuredScoreNet (neuron-major + fp8).

Measured 66.9-68.0us HW exec (baseline 78.3us), rel err 1.236e-2 (< 2e-2).

Computes s(z) = -grad_z U(z) for
  U(z) = sum_k MLP_k(z_k) + sum_r z_8^T W_r z_{8-r}
z: (8192, 9*256) f32, data-parallel over 8 NeuronCores (1024 samples each).

v2 design: every gradient matmul is emitted with the OUTPUT in neuron-major
orientation ([neuron partitions, sample free-dim]) so the W matrices are the
PE-stationary operand (loaded once, streamed against 512-sample tiles):
 - d_lag (8 blocks): fp8e4 DoubleRow matmuls -- W_r stationary with K=256
   (both neuron halves of z_future per pass), z_future pairs streamed.
   2x fewer PE cycles than fp16.  Measured end-to-end rel err 1.24e-2
   (budget 2e-2); the fp8 path is exact-deterministic vs the CPU model.
 - d_fut (block 8): fp16 (fp8 here would cost 3.1e-2 rel err), as one
   K=2048 GEMM over the 16 lag z-chunks, W^T tiles stationary.
 - MLP forward/backward: same h-major scheme as v1 (col-tiled u1, block-diag
   u2 / dh1 with -gW3 folded); d_z contributions are K=32 row-positioned
   matmuls accumulating into the same neuron-major PSUM banks.
All cross/dz weights carry a x64 scale (keeps fp8 W normals); the kernel
stores 64*score in fp16 and the host divides by 64 after the gather.
z ships fp16 for lag blocks (8192 cols/tile) + fp8 pairs for z_future;
output ships fp16.  ~11.7MB HBM traffic/core vs 17.7MB in v1.
"""

import numpy as np
import ml_dtypes

import concourse.bass as bass
import concourse.tile as tile
from concourse import bacc, mybir
from concourse.bass_utils import run_bass_kernel_spmd

AF = mybir.ActivationFunctionType
F32 = mybir.dt.float32
F16 = mybir.dt.float16
F8 = mybir.dt.float8e4
DR = mybir.MatmulPerfMode.DoubleRow

N_CORES = 8
BATCH = 8192
B_CORE = BATCH // N_CORES     # 1024
BT = 512                      # batch tile (PSUM free-dim max for f32)
NBT = B_CORE // BT            # 2 batch tiles per core
NB = 9                        # blocks
P_MAX = 8
NN = 256                      # neurons per block
D = NB * NN                   # 2304
H = 32
NZC = 2 * P_MAX               # fp16 zt chunks per batch tile (lag blocks)
ZW = NZC * BT                 # 8192

# pa (fp16 params) column layout
OFF_U1 = 0                    # 16 chunks x 32 cols (u1 lhsT, blocks 0-7)
OFF_W2 = 16 * 32              # 512: 3 x 128 block-diag u2 weights
OFF_W2T = OFF_W2 + 3 * 128    # 896: dh1 weights, -gW3 folded
OFF_W1T = OFF_W2T + 3 * 128   # 1280: 18 x 128 dz weights (x64)
PW = OFF_W1T + 18 * 128       # 3584

WSCALE = 64.0                 # fp8/cross weight scale, undone on host
MM_MODE = "fp16"              # unused; kept for test.py compat
WARMUP_MMS = 16


def _body(tc, out, zt, z8p, wd, wf, pa, biases, ctx):
    nc = tc.nc

    const = ctx.enter_context(tc.tile_pool(name="const", bufs=1))
    ztp = ctx.enter_context(tc.tile_pool(name="ztp", bufs=2))
    mlpp = ctx.enter_context(tc.tile_pool(name="mlpp", bufs=3, space="PSUM"))
    actp = ctx.enter_context(tc.tile_pool(name="actp", bufs=8))
    du1p = ctx.enter_context(tc.tile_pool(name="du1p", bufs=6))
    outp = ctx.enter_context(tc.tile_pool(name="outp", bufs=5, space="PSUM"))
    outsp = ctx.enter_context(tc.tile_pool(name="outs", bufs=2))

    # ---- inputs -> SBUF, ordered by first use
    pa_sb = const.tile([128, PW], F16, name="pa")
    nc.sync.dma_start(pa_sb[:, 0:OFF_W1T], pa[:, 0:OFF_W1T])
    bias_sb = const.tile([128, 6], F32, name="biassb")
    nc.sync.dma_start(bias_sb[:], biases[:])
    zt_sb = [ztp.tile([128, ZW], F16, tag="zt", name="ztsb") for _ in range(NBT)]
    nc.sync.dma_start(zt_sb[0][:, 0:8 * BT], zt[0, :, 0:8 * BT])
    wd_sb = const.tile([128, 2, 17, 128], F8, name="wd")
    nc.sync.dma_start(wd_sb[:], wd[:])
    z8_sb = [const.tile([128, 2, BT], F8, name=f"z8sb{i}") for i in range(NBT)]
    nc.sync.dma_start(z8_sb[0][:], z8p[0])
    nc.sync.dma_start(z8_sb[1][:], z8p[1])
    wf_sb = const.tile([128, 16, 2, 128], F16, name="wf")
    nc.sync.dma_start(wf_sb[:], wf[:])
    nc.sync.dma_start(zt_sb[0][:, 8 * BT:ZW], zt[0, :, 8 * BT:ZW])
    nc.sync.dma_start(pa_sb[:, OFF_W1T:PW], pa[:, OFF_W1T:PW])
    for a, b in [(0, 8 * BT), (8 * BT, ZW)]:
        nc.sync.dma_start(zt_sb[1][:, a:b], zt[1, :, a:b])

    outs = [outsp.tile([128, 18 * BT], F16, tag="outs", name="outst")
            for _ in range(NBT)]

    # ---- HAM warm-up on a memset tile: PE busy before any DMA lands
    scr2 = const.tile([128, 8], F16, name="scr2")
    wusrc = const.tile([128, BT], F16, name="wusrc")
    nc.gpsimd.memset(wusrc[:], 0.0)
    wu = outp.tile([128, BT], F32, tag="outp", name="wut")
    for _ in range(WARMUP_MMS):
        nc.tensor.matmul(wu[:], wusrc[:, 0:128], wusrc[:], start=True, stop=True)

    def bridge(n):
        # throwaway matmuls emitted into known DMA-wait holes so the PE
        # clock gate (HAM) stays open; they depend only on wusrc
        bw = outp.tile([128, BT], F32, tag="outp", name="bwt")
        for _ in range(n):
            nc.tensor.matmul(bw[:], wusrc[:, 0:128], wusrc[:],
                             start=True, stop=True)

    u1_state = {}
    act_state = {}
    du1_tiles = {}
    bwd_state = {}
    _ce = [0]

    def copy_eng(dst, src):
        # rotate PSUM->SBUF copies between DVE and ACT (2 DVE : 1 ACT --
        # ACT also carries the 9 activations per tile)
        _ce[0] += 1
        if _ce[0] % 3 == 0:
            nc.scalar.activation(dst, src, AF.Copy)
        else:
            nc.vector.tensor_copy(dst, src)

    st = {}

    def mlp_u1(t, g):
        u1 = mlpp.tile([128, BT], F32, tag="mlpp", name="u1t")
        if g < 2:
            P = 128
            for j in range(4):
                k = 4 * g + j
                for hf in range(2):
                    c = 2 * k + hf
                    nc.tensor.matmul(
                        u1[32 * j:32 * j + 32, :],
                        pa_sb[:, OFF_U1 + 32 * c:OFF_U1 + 32 * c + 32],
                        zt_sb[t][:, BT * c:BT * (c + 1)],
                        start=(hf == 0), stop=(hf == 1),
                        tile_position=(0, 32 * j))
        else:
            P = 32
            for hf in range(2):
                nc.tensor.matmul(
                    u1[0:32, :], wd_sb[:, hf:hf + 1, 16:17, 0:32],
                    z8_sb[t][:, hf:hf + 1, :],
                    start=(hf == 0), stop=(hf == 1), tile_position=(0, 0))
        st[(t, g)] = {"u1": u1, "P": P, "sc": 1.0 / WSCALE if g == 2 else 1.0}

    def act_h1(t, g):
        s = st[(t, g)]
        P = s["P"]
        h1 = actp.tile([128, BT], F16, tag="act", name="h1t")
        nc.scalar.activation(h1[:P], s["u1"][:P], AF.Silu,
                             bias=bias_sb[:P, g:g + 1], scale=s["sc"])
        s["h1"] = h1

    def act_sp1(t, g):
        s = st[(t, g)]
        P = s["P"]
        sp1 = actp.tile([128, BT], F16, tag="act", name="sp1t")
        nc.scalar.activation(sp1[:P], s["u1"][:P], AF.Derivative_silu,
                             bias=bias_sb[:P, g:g + 1], scale=s["sc"])
        s["sp1"] = sp1

    def mlp_u2(t, g):
        s = st[(t, g)]
        P = s["P"]
        u2 = mlpp.tile([128, BT], F32, tag="mlpp", name="u2t")
        nc.tensor.matmul(u2[:P], pa_sb[:P, OFF_W2 + 128 * g:OFF_W2 + 128 * g + P],
                         s["h1"][:P], start=True, stop=True)
        s["u2"] = u2

    def act_sp2(t, g):
        s = st[(t, g)]
        P = s["P"]
        sp2 = actp.tile([128, BT], F16, tag="act", name="sp2t")
        nc.scalar.activation(sp2[:P], s["u2"][:P], AF.Derivative_silu,
                             bias=bias_sb[:P, 3 + g:4 + g])
        s["sp2"] = sp2

    def mlp_dh1(t, g):
        s = st[(t, g)]
        P = s["P"]
        dh1 = mlpp.tile([128, BT], F32, tag="mlpp", name="dh1t")
        nc.tensor.matmul(dh1[:P],
                         pa_sb[:P, OFF_W2T + 128 * g:OFF_W2T + 128 * g + P],
                         sp2 := s["sp2"][:P], start=True, stop=True)
        du1 = du1p.tile([128, BT], F16, tag="du1", name="du1t")
        nc.vector.tensor_mul(du1[:P], dh1[:P], s["sp1"][:P])
        du1_tiles.setdefault(t, [None] * 3)[g] = du1

    def mlp_rest(t, g):
        # g1 runs standalone (its zt chunks arrive late): 2 table loads
        act_h1(t, g)
        act_sp1(t, g)
        mlp_u2(t, g)
        act_sp2(t, g)
        mlp_dh1(t, g)

    def dz_mm(t, op, k, nh):
        # MLP d_z for (block k, neuron half nh), accumulated last into op
        strip = 32 * (k % 4)
        cc = OFF_W1T + 128 * (2 * k + nh)
        nc.tensor.matmul(op[:], pa_sb[strip:strip + 32, cc:cc + 128],
                         du1_tiles[t][k // 4][strip:strip + 32, :],
                         start=False, stop=True, tile_position=(strip, 0))

    def dr_mm(t, op, r, ih):
        ri = 2 * (r - 1) + ih
        nc.tensor.matmul(op[:], wd_sb[:, :, ri:ri + 1, :], z8_sb[t][:],
                         start=True, stop=False, perf_mode=DR)

    def fut_mm(t, o8, c, jh):
        nc.tensor.matmul(o8[:], wf_sb[:, c:c + 1, jh:jh + 1, :],
                         zt_sb[t][:, BT * c:BT * (c + 1)],
                         start=(c == 0), stop=False)

    def copy_out(t, op, c):
        copy_eng(outs[t][:, BT * c:BT * (c + 1)], op[:])

    def store(t, c0, c1):
        nc.gpsimd.dma_start(out[t, :, BT * c0:BT * c1],
                            outs[t][:, BT * c0:BT * c1])

    fut_bank = {}

    def cross_sb(t, sb, mid=None):
        # superblock: 2 lags (4 DR matmuls) + 8 interleaved d_fut chain
        # matmuls + 4 dz closers (adjacent lags -> alternating row strips
        # run pairwise-concurrent) + 4 copies.  The fut chain keeps the PE
        # streaming while DVE/ACT drain the lag banks.
        r0, r1 = 8 - 2 * sb, 7 - 2 * sb
        jh, ch0 = sb // 2, 8 * (sb % 2)
        if sb % 2 == 0:
            fut_bank[(t, jh)] = outp.tile([128, BT], F32, tag="outp",
                                          name="futpt")
        o8 = fut_bank[(t, jh)]
        ops = {}
        for r in (r0, r1):
            for ih in range(2):
                ops[(r, ih)] = outp.tile([128, BT], F32, tag="outp",
                                         name="outpt")
                dr_mm(t, ops[(r, ih)], r, ih)
        for c in range(ch0, ch0 + 8):
            fut_mm(t, o8, c, jh)
        if mid is not None:
            mid()
        for ih in range(2):
            for r in (r0, r1):
                dz_mm(t, ops[(r, ih)], 8 - r, ih)
        for r in (r0, r1):
            for ih in range(2):
                copy_out(t, ops[(r, ih)], 2 * (8 - r) + ih)
        if sb % 2 == 1:
            dz_mm(t, o8, 8, jh)
            copy_out(t, o8, 16 + jh)

    # ---- schedule: acts batched by function (Silu<->Dsilu switch costs a
    # 1.54us ACT table load); groups 0+2 pipeline together, g1 rides the
    # superblock mids when its zt chunks land.
    nc.scalar.activation(scr2[:, 0:8], wusrc[:, 0:8], AF.Silu)
    mlp_u1(0, 0)
    mlp_u1(0, 2)
    bridge(5)
    act_h1(0, 0)
    act_h1(0, 2)
    act_sp1(0, 0)
    act_sp1(0, 2)
    mlp_u2(0, 0)
    mlp_u2(0, 2)
    act_sp2(0, 0)
    act_sp2(0, 2)
    mlp_dh1(0, 0)
    mlp_dh1(0, 2)
    cross_sb(0, 0)
    cross_sb(0, 1, mid=lambda: mlp_u1(0, 1))
    store(0, 0, 8)
    store(0, 16, 17)
    cross_sb(0, 2, mid=lambda: mlp_rest(0, 1))
    cross_sb(0, 3)
    store(0, 8, 16)
    mlp_u1(1, 0)
    mlp_u1(1, 2)
    bridge(6)
    act_h1(1, 0)
    act_h1(1, 2)
    act_sp1(1, 0)
    act_sp1(1, 2)
    mlp_u2(1, 0)
    mlp_u2(1, 2)
    act_sp2(1, 0)
    act_sp2(1, 2)
    mlp_dh1(1, 0)
    mlp_dh1(1, 2)
    store(0, 17, 18)
    cross_sb(1, 0, mid=lambda: mlp_u1(1, 1))
    cross_sb(1, 1, mid=lambda: mlp_rest(1, 1))
    store(1, 0, 8)
    store(1, 16, 17)
    cross_sb(1, 2)
    store(1, 8, 12)
    cross_sb(1, 3)
    store(1, 12, 16)
    store(1, 17, 18)


# ------------------------------------------------------------- build + launch

_CACHED = {}


def _build():
    if "v2" in _CACHED:
        return _CACHED["v2"]
    nc = bacc.Bacc("TRN2", target_bir_lowering=False, debug=False,
                   num_devices=N_CORES)
    zt = nc.dram_tensor("zt", [NBT, 128, ZW], F16, kind="ExternalInput").ap()
    z8p = nc.dram_tensor("z8p", [NBT, 128, 2, BT], F8, kind="ExternalInput").ap()
    wd = nc.dram_tensor("wd", [128, 2, 17, 128], F8, kind="ExternalInput").ap()
    wf = nc.dram_tensor("wf", [128, 16, 2, 128], F16, kind="ExternalInput").ap()
    pa = nc.dram_tensor("pa", [128, PW], F16, kind="ExternalInput").ap()
    biases = nc.dram_tensor("biases", [128, 6], F32, kind="ExternalInput").ap()
    out = nc.dram_tensor("out", [NBT, 128, 18 * BT], F16,
                         kind="ExternalOutput").ap()

    from contextlib import ExitStack
    with tile.TileContext(nc) as tc:
        with ExitStack() as ctx:
            _body(tc, out, zt, z8p, wd, wf, pa, biases, ctx)
    nc.compile()
    _CACHED["v2"] = nc
    return nc


def _prep_params(gW1, gb1, gW2, gb2, gW3, gb3, W):
    S = WSCALE
    pa = np.zeros((128, PW), np.float32)
    biases = np.zeros((128, 6), np.float32)
    for k in range(8):
        for hf in range(2):
            c = 2 * k + hf
            pa[:, OFF_U1 + 32 * c:OFF_U1 + 32 * c + 32] = \
                gW1[k, 128 * hf:128 * (hf + 1), :]
    for k in range(NB):
        g, j = k // 4, k % 4
        rs = slice(32 * j, 32 * j + 32)
        pa[rs, OFF_W2 + 128 * g + 32 * j:OFF_W2 + 128 * g + 32 * j + 32] = gW2[k]
        # dh1 lhsT[g', h] = -gW3[g'] * gW2[h, g']
        pa[rs, OFF_W2T + 128 * g + 32 * j:OFF_W2T + 128 * g + 32 * j + 32] = \
            -gW3[k][:, None] * gW2[k].T
        biases[rs, g] = gb1[k]
        biases[rs, 3 + g] = gb2[k]
        strip = 32 * (k % 4)
        for nh in range(2):
            cc = OFF_W1T + 128 * (2 * k + nh)
            pa[strip:strip + 32, cc:cc + 128] = \
                S * gW1[k, 128 * nh:128 * (nh + 1), :].T

    wd = np.zeros((128, 2, 17, 128), np.float32)
    for r in range(1, P_MAX + 1):
        for kt in range(2):
            for ih in range(2):
                wd[:, kt, 2 * (r - 1) + ih, :] = \
                    -S * W[r - 1][128 * kt:128 * (kt + 1), 128 * ih:128 * (ih + 1)]
    for kt in range(2):
        wd[:, kt, 16, 0:32] = S * gW1[8, 128 * kt:128 * (kt + 1), :]

    wf = np.zeros((128, 16, 2, 128), np.float32)
    for c in range(16):
        bc, hc = c // 2, c % 2
        for jh in range(2):
            wf[:, c, jh, :] = \
                -S * W[7 - bc][128 * jh:128 * (jh + 1),
                               128 * hc:128 * (hc + 1)].T
    return {"pa": pa.astype(np.float16),
            "wd": wd.astype(ml_dtypes.float8_e4m3),
            "wf": wf.astype(np.float16),
            "biases": biases}


def run(inputs, trace=False):
    nc = _build()
    params = _prep_params(
        np.asarray(inputs["gW1"]), np.asarray(inputs["gb1"]),
        np.asarray(inputs["gW2"]), np.asarray(inputs["gb2"]),
        np.asarray(inputs["gW3"]), np.asarray(inputs["gb3"]),
        np.asarray(inputs["W"]))
    z = np.asarray(inputs["z"])
    in_maps = []
    for ci in range(N_CORES):
        zc = z[ci * B_CORE:(ci + 1) * B_CORE]
        # zt[t, p, BT*c + s] = zc[BT*t + s, 128c + p] for lag blocks
        ztc = np.ascontiguousarray(
            zc[:, :NZC * 128].reshape(NBT, BT, NZC, 128).transpose(0, 3, 2, 1)
        ).reshape(NBT, 128, ZW).astype(np.float16)
        # z8p[t, p, kt, s] = zc[BT*t + s, 2048 + 128*kt + p]
        z8c = np.ascontiguousarray(
            zc[:, NZC * 128:].reshape(NBT, BT, 2, 128).transpose(0, 3, 2, 1)
        ).astype(ml_dtypes.float8_e4m3)
        in_maps.append({"zt": ztc, "z8p": z8c, **params})
    res = run_bass_kernel_spmd(nc, in_maps, core_ids=list(range(N_CORES)),
                               trace=trace)
    outs = []
    for r in res.results:
        o = r["out"].astype(np.float32) / WSCALE
        outs.append(o.reshape(NBT, 128, 18, BT).transpose(0, 3, 2, 1)
                    .reshape(B_CORE, D))
    return np.concatenate(outs, axis=0), res


def kernel(**inputs) -> np.ndarray:
    out, _ = run(inputs, trace=False)
    return out
